# revision 6
# baseline (speedup 1.0000x reference)
"""Cross-attention kernel for Trainium2, 8 NeuronCores.

Problem (hardcoded): x[4,2048,1024], cond[4,1024,768], dim=1024, cond_dim=768,
H=16 heads, hd=64.  out = proj(softmax(q k^T / sqrt(hd)) v) + proj_b.

Sharding: Megatron-style hybrid — batch (4) x head-half (2) = 8 shards.
Core c handles batch b=c//2 and heads [8*(c%2), 8*(c%2)+8).  Each core
computes its 8 heads' attention and a partial projection output [2048,1024];
the host sums the two partials per batch and adds the biases folded out of
the device program (proj_b and the v-bias term kv_b_v @ proj_w).

All matmuls run in bf16 (fp32 PSUM accumulate): same 1-cycle/row PE rate as
fp32r but Fast Weight Load stays enabled (fp32 weights disable it via the
LastMatmultFP32HI guard), so LDWEIGHTS hides behind the matmul stream, and
DMA/SBUF/DVE traffic halves.

Schedule: one flat software-pipelined loop over 128 (span, pair, mc) steps.
Each step issues the QK pair (two K=64 matmuls on disjoint PE row groups ->
concurrent), the exp ACT op, deadline-driven filler matmuls (kT/v/q
projections and the previous span's output projection), and trailing AV
matmuls (3 cond-chunks behind exp).  AV flushes weave across pair
boundaries so the ACT stream never waits on a flush burst.  The scalar
engine's exp stream (128 x ~1.15us) is the critical path in steady state;
everything else hides under it.
"""

import sys

if '/opt/trn_rl_repo' not in sys.path:
    sys.path.insert(0, '/opt/trn_rl_repo')

import numpy as np

B, N, C = 4, 2048, 1024
CONDN, CONDC = 1024, 768
H, HD = 16, 64
N_CORES = 8
SCALE = HD ** -0.5

NPAIR = 4            # head pairs per core (8 heads)
NSPAN = 4            # query spans of 512
SPAN = 512
NMC = 8              # cond chunks of 128
KC_Q = 8             # contraction chunks for q proj (1024/128)
KC_KV = 6            # contraction chunks for kv proj (768/128)
NSTEP = NSPAN * NPAIR * NMC

_COMPILED = None
LAST_RESULTS = None


def _build():
    import concourse.bacc as bacc
    import concourse.mybir as mybir
    from concourse import tile

    BF16 = mybir.dt.bfloat16
    F32 = mybir.dt.float32
    MULT = mybir.AluOpType.mult
    EXP = mybir.ActivationFunctionType.Exp

    nc = bacc.Bacc("TRN2", target_bir_lowering=False, num_devices=N_CORES)

    xT_d = nc.dram_tensor("xT", [C, N], BF16, kind="ExternalInput")
    condT_d = nc.dram_tensor("condT", [CONDC, CONDN], BF16, kind="ExternalInput")
    qw_d = nc.dram_tensor("qw", [C, 512], BF16, kind="ExternalInput")
    kwk_d = nc.dram_tensor("kwk", [CONDC, 512], BF16, kind="ExternalInput")
    kwv_d = nc.dram_tensor("kwv", [CONDC, 512], BF16, kind="ExternalInput")
    pw_d = nc.dram_tensor("pw", [512, C], BF16, kind="ExternalInput")
    qb_d = nc.dram_tensor("qb", [512], F32, kind="ExternalInput")
    kb_d = nc.dram_tensor("kb", [512], F32, kind="ExternalInput")
    out_d = nc.dram_tensor("out", [N, C], BF16, kind="ExternalOutput")

    with tile.TileContext(nc) as tc:
        with (
            tc.tile_pool(name="const", bufs=1) as const,
            tc.tile_pool(name="xt", bufs=16) as xt_pool,
            tc.tile_pool(name="ct", bufs=12) as ct_pool,
            tc.tile_pool(name="qt", bufs=2) as qt_pool,
            tc.tile_pool(name="ex", bufs=7) as ex_pool,
            tc.tile_pool(name="ot", bufs=2) as ot_pool,
            tc.tile_pool(name="rc", bufs=2) as rc_pool,
            tc.tile_pool(name="ob", bufs=3) as ob_pool,
            tc.tile_pool(name="psA", bufs=2, space="PSUM") as psA,
            tc.tile_pool(name="psB", bufs=2, space="PSUM") as psB,
            tc.tile_pool(name="psC", bufs=2, space="PSUM") as psC,
        ):
            # ---- persistent tiles --------------------------------------------
            kwk_sb = const.tile([128, KC_KV, 512], BF16)
            kwv_sb = const.tile([128, KC_KV, 512], BF16)
            kb_sb = const.tile([128, NPAIR], F32)
            qw_sb = const.tile([128, KC_Q, 512], BF16)
            qb_sb = const.tile([128, NPAIR], F32)
            pw_sb = const.tile([128, NPAIR, C], BF16)
            kT_sb = const.tile([128, NPAIR, NMC, 128], BF16)
            vones = const.tile([128, NMC, 8, 128], BF16)

            # [v|1]/[1|v] interleave: the ones columns are constant; set them
            # on the (otherwise idle) DVE instead of DMAing a 2MB constant.
            nc.vector.memset(vones[:, :, 0::2, 64:128], 1.0)
            nc.vector.memset(vones[:, :, 1::2, 0:64], 1.0)

            # ---- DMA prologue: minimal critical path to the first exp --------
            # kwk/ct0 interleaved per contraction chunk (the first kT matmul
            # starts after ~256KB), then q weights for pair 0 only, x span 0,
            # then everything else in consumption order.
            kwk_r = kwk_d.ap().rearrange("(kc p) m -> p kc m", p=128)
            cts = {0: [], 1: []}
            for kc in range(KC_KV):
                nc.sync.dma_start(kwk_sb[:, kc], kwk_r[:, kc])
                ct = ct_pool.tile([128, 512], BF16, name="ct")
                nc.sync.dma_start(ct[:], condT_d.ap()[kc * 128:(kc + 1) * 128, 0:512])
                cts[0].append(ct)
            nc.sync.dma_start(kb_sb[:], kb_d.ap().rearrange("(pp p) -> p pp", p=128))

            # ---- work-unit emitters ------------------------------------------
            def kt_group(p, ms):
                cell = {}

                def mm(kc):
                    if kc == 0:
                        cell["ps"] = psC.tile([128, 512], F32, tag="C", name="kps")
                    nc.tensor.matmul(
                        cell["ps"][:], kwk_sb[:, kc, p * 128:(p + 1) * 128],
                        cts[ms][kc][:],
                        start=(kc == 0), stop=(kc == KC_KV - 1),
                    )
                    if kc == KC_KV - 1:
                        nc.vector.tensor_scalar_add(
                            kT_sb[:, p, ms * 4:(ms + 1) * 4, :], cell["ps"][:],
                            kb_sb[:, p:p + 1],
                        )

                return [lambda kc=kc: mm(kc) for kc in range(KC_KV)]

            def v_group(mc):
                ms, mj = mc // 4, mc % 4
                cell = {}

                def mm(kc):
                    if kc == 0:
                        cell["ps"] = psC.tile([128, 512], F32, tag="C", name="vps")
                    nc.tensor.matmul(
                        cell["ps"][:], cts[ms][kc][:, mj * 128:(mj + 1) * 128],
                        kwv_sb[:, kc, :],
                        start=(kc == 0), stop=(kc == KC_KV - 1),
                    )
                    if kc == KC_KV - 1:
                        ps_v = cell["ps"].rearrange("q (h d) -> q h d", d=64)
                        nc.vector.tensor_copy(vones[:, mc, 0::2, 0:64], ps_v[:, 0::2, :])
                        nc.vector.tensor_copy(vones[:, mc, 1::2, 64:128], ps_v[:, 1::2, :])

                return [lambda kc=kc: mm(kc) for kc in range(KC_KV)]

            def qproj_pair(qt, xts, p):
                cell = {}

                def mm(kc):
                    if kc == 0:
                        cell["ps"] = psC.tile([128, 512], F32, tag="C", name="qps")
                    nc.tensor.matmul(
                        cell["ps"][:], qw_sb[:, kc, p * 128:(p + 1) * 128], xts[kc][:],
                        start=(kc == 0), stop=(kc == KC_Q - 1),
                    )
                    if kc == KC_Q - 1:
                        nc.vector.tensor_scalar_add(
                            qt[:, p, :], cell["ps"][:], qb_sb[:, p:p + 1],
                        )

                return [lambda kc=kc: mm(kc) for kc in range(KC_Q)]

            def proj_group(s, ot, t, o, pool=None):
                cell = {}
                pool_ = pool or psC

                def mm(p):
                    if p == 0:
                        if pool_ is psA:
                            big = psA.tile([128, 1024], F32, tag="A", name="ppA")
                            cell["pp"] = big[:, 0:512]
                        else:
                            cell["pp"] = psC.tile([128, 512], F32, tag="C", name="pp")[:]
                    nc.tensor.matmul(
                        cell["pp"], ot[:, p, t * 128:(t + 1) * 128],
                        pw_sb[:, p, o * 512:(o + 1) * 512],
                        start=(p == 0), stop=(p == NPAIR - 1),
                    )
                    if p == NPAIR - 1:
                        ob = ob_pool.tile([128, 512], BF16, name="ob")
                        nc.vector.tensor_copy(ob[:], cell["pp"])
                        nc.sync.dma_start(
                            out_d.ap()[s * SPAN + t * 128:s * SPAN + (t + 1) * 128,
                                       o * 512:(o + 1) * 512],
                            ob[:],
                        )

                return [lambda p=p: mm(p) for p in range(NPAIR)]

            def dma_xts(s):
                xts = []
                for kc in range(KC_Q):
                    xt = xt_pool.tile([128, 512], BF16, name="xt")
                    nc.sync.dma_start(
                        xt[:],
                        xT_d.ap()[kc * 128:(kc + 1) * 128, s * SPAN:(s + 1) * SPAN],
                    )
                    xts.append(xt)
                return xts

            # ---- lead-in: pair 0's kT (cond span 0) + q tile -----------------
            for th in kt_group(0, 0):
                th()
            qw_r = qw_d.ap().rearrange("(kc p) m -> p kc m", p=128)
            nc.sync.dma_start(qw_sb[:, :, 0:128], qw_r[:, :, 0:128])
            nc.sync.dma_start(qb_sb[:], qb_d.ap().rearrange("(pp p) -> p pp", p=128))
            xts0 = dma_xts(0)
            qts = {0: qt_pool.tile([128, NPAIR, SPAN], BF16, name="qt")}
            for th in qproj_pair(qts[0], xts0, 0):
                th()

            # remaining bulk DMA, in consumption order
            nc.sync.dma_start(kwv_sb[:], kwv_d.ap().rearrange("(kc p) m -> p kc m", p=128))
            for kc in range(KC_KV):
                ct = ct_pool.tile([128, 512], BF16, name="ct")
                nc.sync.dma_start(ct[:], condT_d.ap()[kc * 128:(kc + 1) * 128, 512:1024])
                cts[1].append(ct)
            nc.sync.dma_start(qw_sb[:, :, 128:512], qw_r[:, :, 128:512])

            # ---- global filler schedule (deadline-ordered thunk list) --------
            fillers = []  # (deadline_step, thunk)

            def add(dl, thunks):
                for t in thunks:
                    fillers.append((dl, t))

            xts = {0: xts0}
            ots = {}

            add(3, kt_group(0, 1))
            add(4, v_group(0))
            add(5, v_group(1))
            add(6, v_group(2))
            add(7, kt_group(1, 0))
            add(7, qproj_pair(qts[0], xts0, 1))
            add(7, v_group(3))
            add(8, v_group(4))
            add(9, v_group(5))
            add(10, v_group(6))
            add(11, v_group(7))
            add(11, kt_group(1, 1))
            add(15, kt_group(2, 0))
            add(15, qproj_pair(qts[0], xts0, 2))
            add(19, kt_group(2, 1))
            add(23, kt_group(3, 0))
            add(23, qproj_pair(qts[0], xts0, 3))
            add(27, kt_group(3, 1))

            def build_span_fillers(s):
                """Called at the start of span s: queue q-proj for upcoming
                spans and the output projection of span s-1."""
                base = s * 32
                if s == 1:
                    nc.sync.dma_start(
                        pw_sb[:], pw_d.ap().rearrange("(pp p) o -> p pp o", p=128))
                if s + 1 < NSPAN:
                    # q-proj of span s+1 (for s=2 also the tail of span 3's)
                    if s + 1 == 3:
                        pass  # span 3's qproj is split across spans 2 and 3
                    xts[s + 1] = dma_xts(s + 1)
                    qts[s + 1] = qt_pool.tile([128, NPAIR, SPAN], BF16, name="qt")
                    if s + 1 < 3:
                        for p in range(NPAIR):
                            add(base + 31 + 8 * p, qproj_pair(qts[s + 1], xts[s + 1], p))
                    else:
                        for p in range(2):
                            add(base + 24 + 4 * p, qproj_pair(qts[3], xts[3], p))
                        for p in (2, 3):
                            add(base + 32 + 8 * p - 1, qproj_pair(qts[3], xts[3], p))
                if s >= 1:
                    idx = 0
                    for t in range(4):
                        for o in range(2):
                            add(base + 8 + idx * 3, proj_group(s - 1, ots[s - 1], t, o))
                            idx += 1

            fill_pos = [0]

            def emit_fillers(step):
                remaining = len(fillers) - fill_pos[0]
                steps_left = max(1, NSTEP - step)
                quota = -(-remaining // steps_left)
                n = 0
                while fill_pos[0] < len(fillers) and (
                    fillers[fill_pos[0]][0] <= step + 3 or n < quota
                ):
                    fillers[fill_pos[0]][1]()
                    fill_pos[0] += 1
                    n += 1

            # ---- flat attention loop -----------------------------------------
            pend = []          # (s, p, mc, ex_tile)
            pair_av = {}       # (s, p) -> [av0, av1]

            def emit_av(s, p, mc, ex, final_pair):
                if mc == 0:
                    pair_av[(s, p)] = [
                        psB.tile([128, 512], F32, tag="av", name=f"av{h}")
                        for h in range(2)
                    ]
                av = pair_av[(s, p)]
                for h in range(2):
                    nc.tensor.matmul(
                        av[h][:], vones[:, mc, 2 * p + h, :], ex[:, h, :],
                        start=(mc == 0), stop=(mc == NMC - 1),
                    )
                if mc == NMC - 1:
                    emit_normalize(s, p, final_pair)

            def emit_normalize(s, p, final_pair):
                av = pair_av.pop((s, p))
                ot = ots[s]
                sums = rc_pool.tile([128, 512], F32, name="sums")
                if final_pair:
                    # ACT is idle after the last exp: split the two sums
                    # copies across ACT and DVE to shorten the tail chain.
                    nc.scalar.copy(sums[0:64, :], av[0][64:128, :])
                else:
                    nc.vector.tensor_copy(sums[0:64, :], av[0][64:128, :])
                nc.vector.tensor_copy(sums[64:128, :], av[1][0:64, :])
                rcp = rc_pool.tile([128, 512], F32, name="rcp")
                nc.vector.reciprocal_approx_fast(rcp[:], sums[:])
                nc.vector.tensor_tensor(
                    ot[0:64, p, :], av[0][0:64, :], rcp[0:64, :], op=MULT,
                )
                nc.vector.tensor_tensor(
                    ot[64:128, p, :], av[1][64:128, :], rcp[64:128, :], op=MULT,
                )

            for g in range(NSTEP):
                s, rem = divmod(g, 32)
                p, mc = divmod(rem, NMC)
                if rem == 0:
                    ots[s] = ot_pool.tile([128, NPAIR, SPAN], BF16, name="ot")
                    build_span_fillers(s)
                qt = qts[s]
                qk = psA.tile([128, 1024], F32, tag="A", name="qk")
                qk2 = qk.rearrange("q (h n) -> q h n", h=2)
                nc.tensor.matmul(
                    qk2[:, 0], kT_sb[0:64, p, mc, :], qt[0:64, p, :],
                    start=True, stop=True,
                )
                nc.tensor.matmul(
                    qk2[:, 1], kT_sb[64:128, p, mc, :], qt[64:128, p, :],
                    start=True, stop=True,
                )
                ex = ex_pool.tile([128, 2, 512], BF16, name="ex")
                nc.scalar.activation(ex[:], qk2[:], EXP, scale=SCALE)
                pend.append((s, p, mc, ex))
                emit_fillers(g)
                trail = 5 if g < 6 else (4 if g < 8 else 3)
                while len(pend) > trail:
                    ps_, pp_, pmc_, pex_ = pend.pop(0)
                    emit_av(ps_, pp_, pmc_, pex_, False)

            # ---- tail --------------------------------------------------------
            # Flush the last pair's AVs, keep the PE busy through its
            # normalize with the last span's proj partials (pairs 0-2), then
            # finish each group with its pair-3 matmul.
            ps_, pp_, pmc_, pex_ = pend.pop(0)
            emit_av(ps_, pp_, pmc_, pex_, False)
            groups = []
            for idx, (t, o) in enumerate([(t, o) for t in range(4) for o in range(2)]):
                pool = psA if idx in (2, 3) else psC
                groups.append(proj_group(NSPAN - 1, ots[NSPAN - 1], t, o, pool=pool))
            for th in groups[0][:3]:
                th()
            ps_, pp_, pmc_, pex_ = pend.pop(0)
            emit_av(ps_, pp_, pmc_, pex_, False)
            for th in groups[1][:3]:
                th()
            ps_, pp_, pmc_, pex_ = pend.pop(0)
            emit_av(ps_, pp_, pmc_, pex_, True)   # includes final normalize
            for gi in (2, 3):
                for th in groups[gi][:3]:
                    th()
            for gi in range(4):
                groups[gi][3]()
            for gi in range(4, 8):
                for th in groups[gi]:
                    th()

    nc.compile()
    return nc


def _get_compiled():
    global _COMPILED
    if _COMPILED is None:
        _COMPILED = _build()
    return _COMPILED


def kernel(x, cond, q_w, q_b, kv_w, kv_b, proj_w, proj_b):
    global LAST_RESULTS
    import ml_dtypes
    from concourse.bass_utils import run_bass_kernel_spmd

    BF = ml_dtypes.bfloat16
    x = np.asarray(x, np.float32)
    cond = np.asarray(cond, np.float32)
    q_w = np.asarray(q_w, np.float32)
    q_b = np.asarray(q_b, np.float32)
    kv_w = np.asarray(kv_w, np.float32)
    kv_b = np.asarray(kv_b, np.float32)
    proj_w = np.asarray(proj_w, np.float32)
    proj_b = np.asarray(proj_b, np.float32)

    nc = _get_compiled()

    in_maps = []
    for c in range(N_CORES):
        b, hh = c // 2, c % 2
        cs = slice(hh * 512, (hh + 1) * 512)
        in_maps.append({
            "xT": np.ascontiguousarray(x[b].T).astype(BF),
            "condT": np.ascontiguousarray(cond[b].T).astype(BF),
            "qw": np.ascontiguousarray(q_w[:, cs]).astype(BF),
            "kwk": np.ascontiguousarray(kv_w[:, hh * 512:(hh + 1) * 512]).astype(BF),
            "kwv": np.ascontiguousarray(kv_w[:, C + hh * 512:C + (hh + 1) * 512]).astype(BF),
            "pw": np.ascontiguousarray(proj_w[cs, :]).astype(BF),
            "qb": np.ascontiguousarray(q_b[cs]),
            "kb": np.ascontiguousarray(kv_b[hh * 512:(hh + 1) * 512]),
        })

    res = run_bass_kernel_spmd(nc, in_maps, core_ids=list(range(N_CORES)))
    LAST_RESULTS = res

    # host reduction: sum the two head-half partials per batch, add the
    # folded biases (proj_b and the v-bias contribution kv_b_v @ proj_w).
    bias = proj_b.astype(np.float64) + kv_b[C:].astype(np.float64) @ proj_w.astype(np.float64)
    out = np.empty((B, N, C), np.float32)
    for b in range(B):
        acc = res.results[2 * b]["out"].astype(np.float64)
        acc += res.results[2 * b + 1]["out"].astype(np.float64)
        acc += bias
        out[b] = acc.astype(np.float32)
    return out
